# revision 15
# baseline (speedup 1.0000x reference)
"""Deformable Conv2d (mode=2: channel-split halves, shared weight) on 8 TRN2 cores.

Sharding: core i handles (batch b = i//2, output-row half = i%2).
Per core:
  x[b] full [128, 128, 128] f32 (gather reach needs the whole image)
  offset[b] rows for its half: [18, 64, 128]
  computes out[b][:, half*64:(half+1)*64, :]  with channel layout
  [conv(x_half1); conv(x_half2)] = exactly out channels 0..127.

Device pipeline:
  B) build bf16 transposed table xT [16384, 128] (row = pixel, cols = chans)
     in DRAM via PE transposes; pair rows are adjacent -> one 512B gather
     element covers the 2 x-adjacent bilinear corners.
  C) offset prep: PE-transpose offsets to [wo-part, ho, comp], compute
     floor/frac/valid/clipped indices + 4 corner weights per tap on DVE.
  D) per (tap, 16-ho chunk): indirect DMA gather (4096 descs x 512B),
  E) DVE combine 4 corners -> V[c-free] weighted by per-sample weights,
  F) PE transpose V -> [c, s] + 9-tap blockdiag matmul accumulate in PSUM,
  H) bias add + store.
"""

import numpy as np
import ml_dtypes

import concourse.bacc as bacc
import concourse.bass as bass
import concourse.mybir as mybir
from concourse import tile

F32 = mybir.dt.float32
BF16 = mybir.dt.bfloat16
I32 = mybir.dt.int32

C_BIG = 12582912.0  # 1.5 * 2**23, float32 round-to-int trick
MULT = mybir.AluOpType.mult
ADD = mybir.AluOpType.add
SUB = mybir.AluOpType.subtract
MAXOP = mybir.AluOpType.max
MINOP = mybir.AluOpType.min

H = 128
W = 128
C = 128
K2 = 9
HO = 128
WO = 128


def build_program(hhalf=64, chunk_ho=16):
    """One SPMD program; per-core variation comes entirely from input data."""
    n_chunks = hhalf // chunk_ho
    ncols = hhalf * K2  # prep free size

    nc = bacc.Bacc("TRN2", target_bir_lowering=False)

    xdram = nc.dram_tensor("x", [C, H * W], F32, kind="ExternalInput")
    offdram = nc.dram_tensor("offset", [2 * K2, hhalf * WO], F32, kind="ExternalInput")
    wbddram = nc.dram_tensor("wbd", [C, K2, 128], BF16, kind="ExternalInput")
    hkydram = nc.dram_tensor("hky", [128, ncols], F32, kind="ExternalInput")
    wkxdram = nc.dram_tensor("wkx", [128, K2], F32, kind="ExternalInput")
    biasdram = nc.dram_tensor("bias128", [128, 1], F32, kind="ExternalInput")
    outdram = nc.dram_tensor("out", [128, hhalf, WO], F32, kind="ExternalOutput")

    ident_f32 = nc.inline_tensor(np.eye(128, dtype=np.float32), name="ident_f32")
    ident_bf16 = nc.inline_tensor(
        np.eye(128, dtype=ml_dtypes.bfloat16), name="ident_bf16"
    )
    # permutation matrices for the wrapped-16 idx fold:
    # Er[r][p, m] = 1 iff p == 16*r + (m % 16); psum[m, :] = src[16r + m%16, :]
    er_np = np.zeros((128, 8, 128), dtype=np.float32)
    for r in range(8):
        for m in range(128):
            er_np[16 * r + (m % 16), r, m] = 1.0
    er_const = nc.inline_tensor(er_np.reshape(128, 8 * 128), name="er_const")

    with tile.TileContext(nc) as tc:
        with (
            tc.tile_pool(name="dram", bufs=1, space="DRAM") as drampool,
            tc.tile_pool(name="const", bufs=1) as constpool,
            tc.tile_pool(name="persist", bufs=1) as persist,
            tc.tile_pool(name="xload", bufs=2) as xload,
            tc.tile_pool(name="xtout", bufs=2) as xtout,
            tc.tile_pool(name="tb_psum", bufs=2, space="PSUM") as tb_psum,
        ):
            table = drampool.tile([H * W, C], BF16)

            idf = constpool.tile([128, 128], F32)
            nc.sync.dma_start(idf[:], bass.AP(ident_f32, 0, [[128, 128], [1, 128]]))
            idb = constpool.tile([128, 128], BF16)
            nc.sync.dma_start(idb[:], bass.AP(ident_bf16, 0, [[128, 128], [1, 128]]))
            wbd_sb = constpool.tile([C, K2, 128], BF16)
            nc.sync.dma_start(wbd_sb[:], wbddram[:])
            hky = constpool.tile([128, ncols], F32)
            nc.sync.dma_start(hky[:], hkydram[:])
            wkx = constpool.tile([128, K2], F32)
            nc.sync.dma_start(wkx[:], wkxdram[:])
            bias_sb = constpool.tile([128, 1], F32)
            nc.sync.dma_start(bias_sb[:], biasdram[:])

            # ---- Stage B: transposed bf16 table ----
            for xc in range(16):  # 1024 pixels per chunk
                xin = xload.tile([128, 1024], F32)
                nc.sync.dma_start(xin[:], xdram[:, xc * 1024 : (xc + 1) * 1024])
                xt = xtout.tile([128, 8, 128], BF16)
                for sub in range(8):
                    pt = tb_psum.tile([128, 128], F32, name="pt", tag="pt")
                    nc.tensor.transpose(
                        pt[:], xin[:, sub * 128 : (sub + 1) * 128], idf[:]
                    )
                    nc.scalar.copy(xt[:, sub, :], pt[:])
                # dest rows xc*1024 + sub*128 + p ; iteration (p, sub, c)
                dst = table[xc * 1024 : (xc + 1) * 1024, :].rearrange(
                    "(s p) c -> p s c", s=8, p=128
                )
                nc.sync.dma_start(dst, xt[:])

            # ---- Stage C: offset prep ----
            with (
                tc.tile_pool(name="prep", bufs=1) as prep,
                tc.tile_pool(name="offload", bufs=1) as offload,
                tc.tile_pool(name="prep_psum", bufs=2, space="PSUM") as prep_psum,
            ):
                off = offload.tile([18, hhalf * WO], F32)
                nc.sync.dma_start(off[:], offdram[:])
                p0 = prep.tile([128, hhalf, 18], F32)
                for ho in range(hhalf):
                    pt = prep_psum.tile([128, 18], F32)
                    nc.tensor.transpose(
                        pt[:], off[:, ho * 128 : (ho + 1) * 128], idf[:18, :18]
                    )
                    nc.scalar.copy(p0[:, ho, :], pt[:])

                _tmp_n = [0]

                def tmp(tagname=None):
                    _tmp_n[0] += 1
                    nm = tagname or f"preptmp{_tmp_n[0]}"
                    return prep.tile([128, ncols], F32, name=nm, tag=nm)

                # views iterating (ho, k): p0 is [128, hhalf, 18], comp = 2k (+1)
                dy = p0[:, :, 0 : 2 * K2 : 2]
                dx = p0[:, :, 1 : 2 * K2 : 2]
                hky_v = hky[:].rearrange("p (h k) -> p h k", h=hhalf, k=K2)
                wkx_v = wkx[:].unsqueeze(1).to_broadcast([128, hhalf, K2])

                def t3(ap_2d):
                    return ap_2d.rearrange("p (h k) -> p h k", h=hhalf, k=K2)

                py = tmp(); nc.vector.tensor_add(t3(py[:]), dy, hky_v)
                t = tmp(); nc.vector.tensor_scalar(t[:], py[:], C_BIG, None, op0=ADD)
                rpy = tmp(); nc.vector.tensor_scalar(rpy[:], t[:], C_BIG, None, op0=SUB)
                m = tmp(); nc.vector.tensor_tensor(m[:], rpy[:], py[:], op=mybir.AluOpType.is_gt)
                y0f = tmp(); nc.vector.tensor_sub(y0f[:], rpy[:], m[:])
                ly = tmp(); nc.vector.tensor_sub(ly[:], py[:], y0f[:])

                px = py; nc.vector.tensor_add(t3(px[:]), dx, wkx_v)
                nc.vector.tensor_scalar(t[:], px[:], C_BIG, None, op0=ADD)
                rpx = rpy; nc.vector.tensor_scalar(rpx[:], t[:], C_BIG, None, op0=SUB)
                nc.vector.tensor_tensor(m[:], rpx[:], px[:], op=mybir.AluOpType.is_gt)
                x0f = tmp(); nc.vector.tensor_sub(x0f[:], rpx[:], m[:])
                lx = tmp(); nc.vector.tensor_sub(lx[:], px[:], x0f[:])

                def valid(src, lo, hi):
                    v = tmp()
                    nc.vector.tensor_scalar(v[:], src[:], lo, None, op0=mybir.AluOpType.is_ge)
                    nc.vector.tensor_scalar(t[:], src[:], hi, None, op0=mybir.AluOpType.is_le)
                    nc.vector.tensor_mul(v[:], v[:], t[:])
                    return v

                vy0 = valid(y0f, 0.0, 127.0)
                vy1 = valid(y0f, -1.0, 126.0)
                vx0 = valid(x0f, 0.0, 127.0)
                vx1 = valid(x0f, -1.0, 126.0)

                y0c = tmp(); nc.vector.tensor_scalar(y0c[:], y0f[:], 0.0, 127.0, op0=MAXOP, op1=MINOP)
                y1c = tmp()
                nc.vector.tensor_scalar(y1c[:], y0f[:], 1.0, 0.0, op0=ADD, op1=MAXOP)
                nc.vector.tensor_scalar(y1c[:], y1c[:], 127.0, None, op0=MINOP)
                xs = tmp(); nc.vector.tensor_scalar(xs[:], x0f[:], 0.0, 126.0, op0=MAXOP, op1=MINOP)

                d = tmp(); nc.vector.tensor_sub(d[:], x0f[:], xs[:])
                eq0 = tmp(); nc.vector.tensor_scalar(eq0[:], d[:], 0.0, None, op0=mybir.AluOpType.is_equal)
                eqm = tmp(); nc.vector.tensor_scalar(eqm[:], d[:], -1.0, None, op0=mybir.AluOpType.is_equal)
                eqp = d; nc.vector.tensor_scalar(eqp[:], d[:], 1.0, None, op0=mybir.AluOpType.is_equal)

                omlx = tmp(); nc.vector.tensor_scalar(omlx[:], lx[:], -1.0, 1.0, op0=MULT, op1=ADD)
                omly = m; nc.vector.tensor_scalar(omly[:], ly[:], -1.0, 1.0, op0=MULT, op1=ADD)

                ax0 = omlx; nc.vector.tensor_mul(ax0[:], omlx[:], vx0[:])
                ax1 = lx; nc.vector.tensor_mul(ax1[:], lx[:], vx1[:])
                ay0 = omly; nc.vector.tensor_mul(ay0[:], omly[:], vy0[:])
                ay1 = ly; nc.vector.tensor_mul(ay1[:], ly[:], vy1[:])

                t4 = vx0; t5 = vx1
                nc.vector.tensor_mul(t4[:], ax0[:], eq0[:])
                nc.vector.tensor_mul(t5[:], ax1[:], eqm[:])
                wl = eqm; nc.vector.tensor_add(wl[:], t4[:], t5[:])
                nc.vector.tensor_mul(t4[:], ax1[:], eq0[:])
                nc.vector.tensor_mul(t5[:], ax0[:], eqp[:])
                wr = eqp; nc.vector.tensor_add(wr[:], t4[:], t5[:])

                # persistent outputs, stored k-major: [128, K2, hhalf]
                wTL = persist.tile([128, K2, hhalf], BF16)
                wTR = persist.tile([128, K2, hhalf], BF16)
                wBL = persist.tile([128, K2, hhalf], BF16)
                wBR = persist.tile([128, K2, hhalf], BF16)

                def kmajor(ap3):  # write view iterating (ho, k)
                    return ap3.transpose([0, 2, 1])

                wtmp = t4
                for wdst, a, b in ((wTL, ay0, wl), (wTR, ay0, wr), (wBL, ay1, wl), (wBR, ay1, wr)):
                    nc.vector.tensor_mul(wtmp[:], a[:], b[:])
                    nc.vector.tensor_copy(kmajor(wdst[:]), t3(wtmp[:]))

                # staged f32 pair-indices, storage order (k, ch, tb, ho')
                nidxcols = K2 * n_chunks * 2 * chunk_ho
                idxf = prep.tile([128, K2, n_chunks, 2, chunk_ho], F32)

                for ch in range(n_chunks):
                    for tb, ysrc in ((0, y0c), (1, y1c)):
                        # dest iterating (ho', k); sources iterate (ho', k) too
                        dstv = idxf[:, :, ch, tb, :].transpose([0, 2, 1])
                        hosl = slice(ch * chunk_ho * K2, (ch + 1) * chunk_ho * K2)
                        src_y = t3(ysrc[:])[:, ch * chunk_ho : (ch + 1) * chunk_ho, :]
                        src_x = t3(xs[:])[:, ch * chunk_ho : (ch + 1) * chunk_ho, :]
                        nc.vector.scalar_tensor_tensor(
                            dstv, src_y, 128.0, src_x, op0=MULT, op1=ADD
                        )

                # fold to wrapped-16 int16 layout via 8 permutation matmuls
                er_sb = constpool.tile([128, 8, 128], F32)
                nc.sync.dma_start(er_sb[:], er_const[:, :].rearrange("p (r m) -> p r m", r=8, m=128))
                idxw = persist.tile([128, nidxcols, 8], mybir.dt.int16)
                idxf2 = idxf[:].rearrange("p a b c d -> p (a b c d)")
                idxw2 = idxw[:]
                nperm = 384 if nidxcols % 384 == 0 else (288 if nidxcols % 288 == 0 else chunk_ho * 2)
                with tc.tile_pool(name="perm_psum", bufs=2, space="PSUM") as perm_psum:
                    for r in range(8):
                        for nb in range(nidxcols // nperm):
                            pp = perm_psum.tile([128, nperm], F32, name="pp", tag="pp")
                            nc.tensor.matmul(
                                pp[:],
                                er_sb[:, r, :],
                                idxf2[:, nb * nperm : (nb + 1) * nperm],
                                start=True,
                                stop=True,
                            )
                            nc.vector.tensor_copy(
                                idxw2[:, nb * nperm : (nb + 1) * nperm, r], pp[:]
                            )

            # ---- Stages D-H ----
            with (
                tc.tile_pool(name="gpool", bufs=2) as gpool,
                tc.tile_pool(name="vpool", bufs=2) as vpool,
                tc.tile_pool(name="ttmp", bufs=2) as ttmp,
                tc.tile_pool(name="rhs", bufs=2) as rhspool,
                tc.tile_pool(name="osb", bufs=2) as osbpool,
                tc.tile_pool(name="ptr_psum", bufs=2, space="PSUM") as ptr_psum,
                tc.tile_pool(name="pout_psum", bufs=2, space="PSUM") as pout_psum,
            ):
                nho_blk = 4  # ho rows per matmul block (N=512)
                for ch in range(n_chunks):
                    c0 = ch * chunk_ho
                    vtile = vpool.tile([128, K2, chunk_ho, 128], BF16)
                    nidx = 2 * chunk_ho * 128
                    # overlapping pair-row view; max idx is 127*128+126, so the
                    # H*W-1 row count keeps the view inside the buffer.
                    table_pairs = bass.AP(
                        table.tensor, table.offset, [[128, H * W - 1], [1, 256]]
                    )
                    for k in range(K2):
                        g = gpool.tile([128, 2 * chunk_ho, 256], BF16)
                        iw0 = (k * n_chunks + ch) * (2 * chunk_ho)
                        nc.gpsimd.dma_gather(
                            g[:],
                            table_pairs,
                            idxw[:, iw0 : iw0 + 2 * chunk_ho, :],
                            nidx,
                            nidx,
                            256,
                            elem_step=128,
                            single_packet=False,
                        )
                        vv = vtile[:, k, :, :]
                        tl = g[:, 0:chunk_ho, 0:128]
                        tr = g[:, 0:chunk_ho, 128:256]
                        bl = g[:, chunk_ho : 2 * chunk_ho, 0:128]
                        br = g[:, chunk_ho : 2 * chunk_ho, 128:256]

                        def wb(wt):
                            return (
                                wt[:, k, c0 : c0 + chunk_ho]
                                .unsqueeze(2)
                                .to_broadcast([128, chunk_ho, 128])
                            )

                        tt_ = ttmp.tile([128, chunk_ho, 128], BF16, tag="combtmp")
                        nc.vector.tensor_tensor(vv, tl, wb(wTL), op=MULT)
                        nc.vector.tensor_tensor(tt_[:], tr, wb(wTR), op=MULT)
                        nc.vector.tensor_add(vv, vv, tt_[:])
                        nc.vector.tensor_tensor(tt_[:], bl, wb(wBL), op=MULT)
                        nc.vector.tensor_add(vv, vv, tt_[:])
                        nc.vector.tensor_tensor(tt_[:], br, wb(wBR), op=MULT)
                        nc.vector.tensor_add(vv, vv, tt_[:])

                    for blk in range(chunk_ho // nho_blk):
                        pout = pout_psum.tile([128, nho_blk * 128], F32)
                        rhs = rhspool.tile([128, nho_blk * 128], BF16)
                        for k in range(K2):
                            for sub in range(nho_blk):
                                ptb = ptr_psum.tile([128, 128], BF16, tag="ptb")
                                nc.tensor.transpose(
                                    ptb[:], vtile[:, k, blk * nho_blk + sub, :], idb[:]
                                )
                                nc.scalar.copy(rhs[:, sub * 128 : (sub + 1) * 128], ptb[:])
                            nc.tensor.matmul(
                                pout[:],
                                wbd_sb[:, k, :],
                                rhs[:],
                                start=(k == 0),
                                stop=(k == K2 - 1),
                            )
                        osb = osbpool.tile([128, nho_blk * 128], F32)
                        nc.vector.tensor_scalar(osb[:], pout[:], bias_sb[:, :1], None, op0=ADD)
                        ho0 = c0 + blk * nho_blk
                        nc.sync.dma_start(
                            outdram[:, ho0 : ho0 + nho_blk, :],
                            osb[:].rearrange("p (h w) -> p h w", h=nho_blk, w=128),
                        )

    nc.compile()
    return nc


def make_inputs(x, offset, weight, bias, core, hhalf=64):
    """Build the per-core in_map (host-side sharding of full inputs)."""
    b, half = core // 2, core % 2
    K2l = 9
    wbd = np.zeros((128, K2l, 128), dtype=np.float32)
    for k in range(K2l):
        wk = weight[:, :, k // 3, k % 3]  # [cout, cin]
        for h in range(2):
            wbd[64 * h : 64 * h + 64, k, 64 * h : 64 * h + 64] = wk.T
    hky = np.zeros((128, hhalf * K2l), dtype=np.float32)
    ho = np.arange(hhalf) + hhalf * half
    kyv = np.repeat(np.arange(3) - 1, 3)  # k//3 - 1
    hky[:, :] = (ho[:, None] + kyv[None, :]).reshape(-1)[None, :]
    wkx = np.zeros((128, K2l), dtype=np.float32)
    kxv = np.tile(np.arange(3) - 1, 3)  # k%3 - 1
    wkx[:, :] = (np.arange(128)[:, None] + kxv[None, :]).astype(np.float32)
    bias128 = np.concatenate([bias, bias]).astype(np.float32).reshape(128, 1)
    return {
        "x": np.ascontiguousarray(x[b].reshape(128, -1), dtype=np.float32),
        "offset": np.ascontiguousarray(
            offset[b][:, hhalf * half : hhalf * (half + 1), :].reshape(18, -1),
            dtype=np.float32,
        ),
        "wbd": wbd.astype(ml_dtypes.bfloat16),
        "hky": hky,
        "wkx": wkx,
        "bias128": bias128,
    }


_NC_CACHE = {}


def kernel(x, offset, weight, bias):
    from concourse.bass_utils import run_bass_kernel_spmd

    if "nc" not in _NC_CACHE:
        _NC_CACHE["nc"] = build_program()
    nc = _NC_CACHE["nc"]
    in_maps = [make_inputs(x, offset, weight, bias, i) for i in range(8)]
    res = run_bass_kernel_spmd(nc, in_maps, core_ids=list(range(8)))
    out = np.zeros((4, 128, 128, 128), dtype=np.float32)
    for i in range(8):
        b, half = i // 2, i % 2
        out[b, :, 64 * half : 64 * (half + 1), :] = res.results[i]["out"]
    return out, np.asarray(offset)
